# revision 53
# baseline (speedup 1.0000x reference)
"""GCN classifier (2x GCNConv + mean-pool + 2-layer MLP) on 8 Trainium2 cores.

Sharding strategy (graph/data parallel per the hint):
- Nodes partitioned contiguously: core c owns dst nodes [c*6250, (c+1)*6250).
- conv1 (aggregate-then-transform): edges + self-loops partitioned by dst
  owner, grouped into 98 windows of 64 dst nodes, padded to 128-edge chunks
  (chunk counts maxed across cores -> one SPMD program). The host ships each
  core its incident edges' x rows pre-scaled by the full sym-norm
  dinv[src]*dinv[dst] and quantized to fp8-e4m3 (chunk-ordered -> one big
  sequential DMA stream per batch of <=4 windows). The scatter-add is
  realized on the PE as matmuls with the fp8 x chunk stationary (FWL) and a
  64-wide 0/1 one-hot (iota-compare on DVE/Pool) as the moving operand,
  accumulating in PSUM -> the aggregation lands feature-major, no
  transposes. Dense W1 applied with fp8 DoubleRow matmuls (both 128-row
  k-tiles in one pass, W1 stationary) + bias + relu -> h1 kept feature-major
  in SBUF only (bf16).
- conv2 + mean-pool fused algebraically: with no nonlinearity between
  conv2's aggregation and the pooling, pooled sums satisfy
  pool[G] = sum_s A[s,G] * (h1[s] @ W2), where
  A[s,G] = dinv[s]*(sum_{e:src=s,dst in G} dinv[dst] + [batch[s]==G]*dinv[s])
  is built on host from edge_index/batch/deg only (structural data). Each
  core computes p = h1 @ W2 (bf16) for its own node chunks and immediately
  accumulates pb^T @ A_chunk into persistent [128,64] PSUM tiles, keeping
  the pooled partials FEATURE-major -- the tail MLP then needs no
  transposes at all.
- One 16KB fp8 AllReduce of the pooled partials at the end (two ARs
  serialize on the CC stream, so a single late one exposes less latency).
  mean+bias+relu and the tiny MLP run replicated in feature-major layout
  (out lands as [DOUT, NG] directly); core 0's output wins.
- Pipelining: per-batch x_edges DMA + one-hot build (triple buffered,
  issue-ahead 2), aggregation/dense/p-chunks interleaved batch by batch so
  the PE stays busy end to end.
"""

import sys
import types

import ml_dtypes
import numpy as np

try:
    import antenv  # noqa: F401

    if "antenv.axon_hooks" not in sys.modules:
        _m = types.ModuleType("antenv.axon_hooks")
        _m._hook = None
        _m.set_axon_ntff_profile_hook = lambda h: setattr(_m, "_hook", h)
        _m.get_axon_ntff_profile_hook = lambda: _m._hook
        sys.modules["antenv.axon_hooks"] = _m
except Exception:
    pass

import concourse.bacc as bacc
import concourse.mybir as mybir
import concourse.tile as tile
from concourse import bass_utils

F32 = mybir.dt.float32
BF16 = mybir.dt.bfloat16
F8 = mybir.dt.float8e4
AF = mybir.ActivationFunctionType
OP = mybir.AluOpType
DR = mybir.MatmulPerfMode.DoubleRow

N = 50000
E = 500000
DIN = 256
DH = 512
NG = 64
DOUT = 16

NCORES = 8
SLICE = N // NCORES  # 6250
WW = 64  # dst window width (one-hot width)
NW = (SLICE + WW - 1) // WW  # 98 windows
NPAD = 6272  # 49 * 128 node columns
NCHK = NPAD // 128  # 49 node chunks
NGRP = 13  # 12 groups of 512 node cols + 1 of 128

# tuning knobs
USE_DR_DENSE = True  # fp8 DoubleRow for the W1 dense
INTERLEAVE_PA = True  # emit pA of group g-1 between dense MMs of group g
CW = DIN + WW  # chunk width in the x_edges stream: 256 x cols + 64 one-hot

_COMPILED: dict = {}


def _group_info(g):
    """(first window, #windows, node col0, #node cols, first chunk, #chunks)"""
    if g < 12:
        return (8 * g, 8, 512 * g, 512, 4 * g, 4)
    return (96, 2, 6144, 128, 48, 1)


def _layout(K1):
    """Batches of <=4 windows: [(g, ws, {w: [(gcol, grel)]}, nch, c0)]."""
    batches = []
    gcol = 0
    for g in range(NGRP):
        w0, nwin, _, _, _, _ = _group_info(g)
        nhalf = 2 if nwin == 8 else 1
        for half in range(nhalf):
            ws = list(range(w0 + half * 4, min(w0 + (half + 1) * 4, w0 + nwin)))
            c0 = gcol
            rel = 0
            wch = {}
            for w in ws:
                lst = []
                for _ in range(int(K1[w])):
                    lst.append((gcol, rel))
                    gcol += 1
                    rel += 1
                wch[w] = lst
            batches.append((g, ws, wch, rel, c0))
    return batches, gcol


def _preprocess(x, edge_index, batch):
    src = np.asarray(edge_index[0], dtype=np.int64)
    dst = np.asarray(edge_index[1], dtype=np.int64)
    batch = np.asarray(batch, dtype=np.int64)

    deg = np.bincount(dst, minlength=N).astype(np.float64) + 1.0
    dinv = (1.0 / np.sqrt(deg)).astype(np.float32)
    cnt = np.maximum(np.bincount(batch, minlength=NG), 1)

    loops = np.arange(N, dtype=np.int64)

    # ---------- conv1: edges + self-loops grouped by (core, 64-window) ----------
    s1 = np.concatenate([src, loops])
    d1 = np.concatenate([dst, loops])
    norm1 = dinv[s1] * dinv[d1]
    core1 = d1 // SLICE
    win1 = (d1 % SLICE) // WW
    key1 = core1 * NW + win1
    order1 = np.argsort(key1, kind="stable")
    ss1, ds1, nn1 = s1[order1], d1[order1], norm1[order1]
    counts1 = np.bincount(key1, minlength=NCORES * NW).reshape(NCORES, NW)
    starts1 = np.zeros(NCORES * NW + 1, dtype=np.int64)
    np.cumsum(counts1.reshape(-1), out=starts1[1:])
    K1 = np.ceil(counts1.max(axis=0) / 128).astype(np.int64)  # [NW]

    meta = tuple(int(v) for v in K1)
    batches, C1 = _layout(K1)

    # ---------- fused conv2+pool coefficient matrix A[s, G] ----------
    gd = batch[dst]
    A = np.bincount(src * NG + gd, weights=dinv[dst].astype(np.float64),
                    minlength=N * NG).reshape(N, NG).astype(np.float32)
    A[loops, batch] += dinv
    A *= dinv[:, None]
    # bake the mean-pool 1/cnt into A, x64 to keep fp8 AR payloads in range;
    # the tail activation divides by 64 via its scale parameter
    A *= (64.0 / np.maximum(cnt, 1).astype(np.float32))[None, :]

    xf = np.asarray(x, np.float32)

    per_core = []
    for c in range(NCORES):
        src_cols = np.zeros((C1, 128), dtype=np.int64)
        nrm_cols = np.zeros((C1, 128), dtype=np.float32)
        dst_cols = np.full((C1, 128), -1.0, dtype=np.float32)
        for _g, ws, wch, _nch, _c0 in batches:
            for w in ws:
                gi = c * NW + w
                e0, e1 = starts1[gi], starts1[gi + 1]
                n_e = int(e1 - e0)
                cols = wch[w]
                k = len(cols)
                sv = np.zeros(k * 128, dtype=np.int64)
                sv[:n_e] = ss1[e0:e1]
                nv = np.zeros(k * 128, dtype=np.float32)
                nv[:n_e] = nn1[e0:e1]
                dv = np.full(k * 128, -1.0, dtype=np.float32)
                dv[:n_e] = (ds1[e0:e1] - (c * SLICE + w * WW)).astype(np.float32)
                for j, (gcol, _r) in enumerate(cols):
                    src_cols[gcol] = sv[j * 128 : (j + 1) * 128]
                    nrm_cols[gcol] = nv[j * 128 : (j + 1) * 128]
                    dst_cols[gcol] = dv[j * 128 : (j + 1) * 128]
        rows = xf[src_cols.reshape(-1)] * nrm_cols.reshape(-1)[:, None]
        combo = np.empty((C1, 128, CW), dtype=ml_dtypes.float8_e4m3)
        combo[:, :, :DIN] = rows.astype(ml_dtypes.float8_e4m3).reshape(C1, 128, DIN)
        combo[:, :, DIN:] = (
            dst_cols[:, :, None] == np.arange(WW, dtype=np.float32)[None, None, :]
        ).astype(ml_dtypes.float8_e4m3)
        x_edges = np.ascontiguousarray(combo.transpose(1, 0, 2)).reshape(128, C1 * CW)

        Ac = np.zeros((NPAD, NG), dtype=np.float32)
        Ac[:SLICE] = A[c * SLICE : (c + 1) * SLICE]
        a_sb = np.ascontiguousarray(
            Ac.reshape(NCHK, 128, NG).transpose(1, 0, 2)
        ).reshape(128, NCHK * NG).astype(ml_dtypes.bfloat16)

        per_core.append(dict(x_edges=x_edges, a_mat=a_sb))
    return meta, per_core, cnt.astype(np.float32)


def _build_program(meta):
    K1 = np.array(meta)
    batches, C1 = _layout(K1)
    max_nch = max(b[3] for b in batches)

    nc = bacc.Bacc("TRN2", target_bir_lowering=False, debug=False, num_devices=NCORES)

    def din(name, shape, dt=F32):
        return nc.dram_tensor(name, shape, dt, kind="ExternalInput").ap()

    x_edges = din("x_edges", [128, C1 * CW], F8)
    a_mat = din("a_mat", [128, NCHK * NG], BF16)
    w1dr = din("w1dr", [128, 2 * DH], F8)  # [p, k(2), m(4), 128] fp8 pairs
    w1bf = din("w1bf", [128, 2 * DH], BF16)  # [p, k(2), fo(512)] bf16 fallback
    # merged bf16 consts: [0:1024] w2b, [1024:1280] wf1, [1280:1296] wf2
    wc_bf = din("wc_bf", [128, 1296], BF16)
    # merged f32 consts: [0:4] b1c, [4:6] b2h, [6:7] bf1c, [7:8] bf2c (rows<16)
    fc32 = din("fc32", [128, 8])
    out = nc.dram_tensor("out", [DOUT, NG], F32, kind="ExternalOutput").ap()

    with tile.TileContext(nc) as tc:
        with (
            tc.tile_pool(name="const", bufs=1) as cp,
            tc.tile_pool(name="big", bufs=1) as bigp,
            tc.tile_pool(name="work", bufs=1) as wp,
            tc.tile_pool(name="psum", bufs=1, space="PSUM") as pp,
            tc.tile_pool(name="dram", bufs=1, space="DRAM") as dp,
        ):
            def load(ap_in, shape, dt=F32, pool=cp):
                t = pool.tile(shape, dt, name=ap_in.tensor.name + "_sb")
                nc.sync.dma_start(t[:], ap_in[:])
                return t

            # loads gating the pipeline start go first; the rest after batch 0

            h1s = [bigp.tile([128, NPAD], BF16, name=f"h1s_{k}") for k in range(4)]

            sfg_groups: dict = {}

            def sfg_of(g):
                # fp8 feature-major conv1 aggregation for group g: [p, k(2), n]
                if g not in sfg_groups:
                    sfg_groups[g] = wp.tile(
                        [128, 2, 512], F8, tag="sfg", bufs=2, name=f"sfg_{g}"
                    )
                return sfg_groups[g]

            # persistent feature-major pool partials: pgo[h] = [128 o, 64 G]
            pgo = [pp.tile([128, NG], F32, name=f"pgo_{h}") for h in range(2)]
            g_local = dp.tile([128, 2 * NG], F8, name="gl")
            g_ag = dp.tile([NCORES * 128, 2 * NG], F8, addr_space="Shared", name="gag")
            gs_all = wp.tile([128, NCORES, 2 * NG], F8, name="gs_all")
            red = wp.tile([128, 2 * NG], F32, name="red")

            def emit_allgather():
                # AllGather the 8 fp8 pool partials (cheaper than AllReduce:
                # no reduce phase on the CC cores) and sum them on the DVE.
                gsb = wp.tile([128, 2, NG], F8, name="gsb")
                for h in range(2):
                    nc.vector.tensor_copy(gsb[:, h, :], pgo[h][:])
                nc.sync.dma_start(g_local[:], gsb[:].rearrange("p h g -> p (h g)"))
                nc.gpsimd.collective_compute(
                    "AllGather",
                    OP.bypass,
                    replica_groups=[list(range(NCORES))],
                    ins=[g_local.opt()],
                    outs=[g_ag.opt()],
                )
                nc.sync.dma_start(
                    gs_all[:],
                    g_ag[:].rearrange("(r p) c -> p r c", r=NCORES),
                )
                # pairwise tree-sum of the 8 partials (contiguous adds beat a
                # strided tensor_reduce on the DVE)
                t4 = wp.tile([128, 4, 2 * NG], F32, name="red4")
                nc.vector.tensor_tensor(
                    out=t4[:], in0=gs_all[:, 0:4, :], in1=gs_all[:, 4:8, :], op=OP.add
                )
                t2 = wp.tile([128, 2, 2 * NG], F32, name="red2")
                nc.vector.tensor_tensor(
                    out=t2[:], in0=t4[:, 0:2, :], in1=t4[:, 2:4, :], op=OP.add
                )
                nc.vector.tensor_tensor(
                    out=red[:], in0=t2[:, 0, :], in1=t2[:, 1, :], op=OP.add
                )

            def emit_stream(bi):
                """One G1 DMA covering one batch; one-hot cols ride along."""
                _g, _ws, _wch, nch, c0 = batches[bi]
                G1 = wp.tile([128, nch, CW], F8, tag="G1", bufs=3, name=f"g1b_{bi}")
                if bi == 0:
                    # split the first transfer so the opening windows land
                    # (and the PE starts) sooner
                    n1 = max(1, nch // 2)
                    nc.sync.dma_start(
                        G1[:, :n1, :].rearrange("p c d -> p (c d)"),
                        x_edges[:, c0 * CW : (c0 + n1) * CW],
                    )
                    nc.sync.dma_start(
                        G1[:, n1:, :].rearrange("p c d -> p (c d)"),
                        x_edges[:, (c0 + n1) * CW : (c0 + nch) * CW],
                    )
                else:
                    nc.sync.dma_start(
                        G1[:].rearrange("p c d -> p (c d)"),
                        x_edges[:, c0 * CW : (c0 + nch) * CW],
                    )
                return G1

            def emit_batch(g, ws, wch, nch, c0, G1):
                sfg = sfg_of(g)
                nw = len(ws)
                wb0 = ws[0] - 8 * g
                pa = pp.tile([128, nw, 2, WW], F32, tag="agg", bufs=2, name=f"pa_{ws[0]}")
                for w in ws:
                    cols = wch[w]
                    wrel = w - ws[0]
                    for j, (_gcol, grel) in enumerate(cols):
                        for h in range(2):
                            nc.tensor.matmul(
                                out=pa[:, wrel, h, :],
                                lhsT=G1[:, grel, h * 128 : (h + 1) * 128],
                                rhs=G1[:, grel, DIN:CW],
                                start=(j == 0),
                                stop=(j == len(cols) - 1),
                            )
                for h in range(2):
                    nc.vector.tensor_copy(
                        sfg[:, h, wb0 * WW : (wb0 + nw) * WW],
                        pa[:, :, h, :],
                    )

            def emit_dense_mm(g, m):
                _, _, n0, ncols, _, _ = _group_info(g)
                sfg = sfg_of(g)
                ph = pp.tile([128, 512], F32, tag="h1", bufs=2, name=f"ph_{g}_{m}")
                if USE_DR_DENSE:
                    nc.tensor.matmul(
                        out=ph[:, :ncols],
                        lhsT=w1_sb[:, :, m, :],
                        rhs=sfg[:, :, :ncols],
                        start=True,
                        stop=True,
                        perf_mode=DR,
                    )
                else:
                    for k in range(2):
                        nc.tensor.matmul(
                            out=ph[:, :ncols],
                            lhsT=w1f_sb[:, k, m * 128 : (m + 1) * 128],
                            rhs=sfg[:, k, :ncols],
                            start=(k == 0),
                            stop=(k == 1),
                        )
                nc.scalar.activation(
                    h1s[m][:, n0 : n0 + ncols], ph[:, :ncols], AF.Relu,
                    bias=fc_sb[:, m : m + 1],
                )

            def emit_pA(cc):
                c0 = cc * 128
                ppm = pp.tile([128, DH // 2], F32, tag="p2", bufs=2, name=f"ppm_{cc}")
                for k in range(4):
                    nc.tensor.matmul(
                        out=ppm[:],
                        lhsT=h1s[k][:, c0 : c0 + 128],
                        rhs=wc_sb[:, k * (DH // 2) : (k + 1) * (DH // 2)],
                        start=(k == 0),
                        stop=(k == 3),
                    )
                pb = wp.tile([128, DH // 2], BF16, tag="pb", bufs=2, name=f"pb_{cc}")
                nc.vector.tensor_copy(pb[:], ppm[:])
                for h in range(2):
                    nc.tensor.matmul(
                        out=pgo[h][:],
                        lhsT=pb[:, h * 128 : (h + 1) * 128],
                        rhs=a_sb[:, cc * NG : (cc + 1) * NG],
                        start=(cc == 0),
                        stop=(cc == NCHK - 1),
                    )

            streams = {0: emit_stream(0), 1: emit_stream(1)}
            if USE_DR_DENSE:
                w1_sb = load(w1dr, [128, 2, 4, 128], F8)
            else:
                w1f_sb = load(w1bf, [128, 2, DH], BF16)
            fc_sb = load(fc32, [128, 8])
            # warm-up collective: absorbs the CC barrier + cold firmware setup
            # during the main phase so the real AllGather launches warm. Kept
            # to a single probe so a slow barrier can't push the chain past
            # the end of the main phase.
            warm_l = dp.tile([128, 128], F8, name="warm_l")
            warm_ag = dp.tile([128 * 8, 128], F8, addr_space="Shared", name="warm_ag")
            nc.gpsimd.collective_compute(
                "AllGather", OP.bypass, replica_groups=[list(range(NCORES))],
                ins=[warm_l.opt()], outs=[warm_ag.opt()],
            )
            bidx = 0
            pending = []
            for g in range(NGRP):
                _, nwin, _, _, cc0, nccs = _group_info(g)
                nb = 2 if nwin == 8 else 1
                for _b in range(nb):
                    if bidx + 2 < len(batches):
                        streams[bidx + 2] = emit_stream(bidx + 2)
                    G1 = streams.pop(bidx)
                    emit_batch(*batches[bidx], G1)
                    bidx += 1
                    if bidx == 1:
                        a_sb = load(a_mat, [128, NCHK * NG], BF16)
                        wc_sb = load(wc_bf, [128, 1296], BF16)
                for m in range(4):
                    emit_dense_mm(g, m)
                    if INTERLEAVE_PA and pending:
                        emit_pA(pending.pop(0))
                if INTERLEAVE_PA:
                    pending.extend(range(cc0, cc0 + nccs))
                else:
                    for cc in range(cc0, cc0 + nccs):
                        emit_pA(cc)
            for cc in pending:
                emit_pA(cc)

            # ---- tail: AllGather + DVE reduce + mean/bias/relu + MLP,
            # all feature-major (1/cnt baked into A on host, x64; /64 here)
            emit_allgather()
            curv = red[:].rearrange("p (h g) -> p h g", h=2)
            grelu = wp.tile([128, 2, NG], BF16, name="grelu")
            for h in range(2):
                nc.scalar.activation(
                    grelu[:, h, :], curv[:, h, :], AF.Relu,
                    bias=fc_sb[:, 4 + h : 5 + h], scale=1.0 / 64.0,
                )
            pz = pp.tile([128, NG], F32, tag="p2", bufs=2, name="pz")
            for k in range(2):
                nc.tensor.matmul(
                    out=pz[:],
                    lhsT=wc_sb[:, 1024 + k * 128 : 1024 + (k + 1) * 128],
                    rhs=grelu[:, k, :],
                    start=(k == 0),
                    stop=(k == 1),
                )
            zsb = wp.tile([128, NG], BF16, name="zsb")
            nc.scalar.activation(zsb[:], pz[:], AF.Relu, bias=fc_sb[:, 6:7])
            po = pp.tile([DOUT, NG], F32, tag="agg", bufs=2, name="po")
            nc.tensor.matmul(
                out=po[:], lhsT=wc_sb[:, 1280:1296], rhs=zsb[:], start=True, stop=True
            )
            osb = wp.tile([DOUT, NG], F32, name="osb")
            nc.scalar.activation(osb[:], po[:], AF.Relu, bias=fc_sb[:16, 7:8])
            nc.sync.dma_start(out[:], osb[:])

    nc.compile()
    return nc


def _get_program(meta):
    if meta not in _COMPILED:
        _COMPILED[meta] = _build_program(meta)
    return _COMPILED[meta]


def _make_in_maps(W1, b1, W2, b2, Wf1, bf1, Wf2, bf2, per_core, cnt, meta):
    bf = ml_dtypes.bfloat16
    f8 = ml_dtypes.float8_e4m3
    W1 = np.asarray(W1, np.float32)
    W2 = np.asarray(W2, np.float32)
    Wf1 = np.asarray(Wf1, np.float32)
    b2 = np.asarray(b2, np.float32)
    K1 = np.array(meta)
    batches, _C1 = _layout(K1)
    max_nch = max(b[3] for b in batches)

    # w1dr[p, k, m, c] = W1[k*128+p, m*128+c]
    w1dr = np.ascontiguousarray(
        W1.reshape(2, 128, 4, 128).transpose(1, 0, 2, 3).reshape(128, 2 * DH)
    )
    w1bf = np.ascontiguousarray(
        W1.reshape(2, 128, DH).transpose(1, 0, 2).reshape(128, 2 * DH)
    )
    w2b = np.ascontiguousarray(
        np.concatenate([W2[k * 128 : (k + 1) * 128, :] for k in range(4)], axis=1)
    )
    wf1b = np.ascontiguousarray(
        Wf1.reshape(2, 128, DH // 4).transpose(1, 0, 2).reshape(128, 2 * (DH // 4))
    )
    wc = np.concatenate([w2b, wf1b, np.asarray(Wf2, np.float32)], axis=1)
    fc = np.zeros((128, 8), np.float32)
    fc[:, 0:4] = np.asarray(b1, np.float32).reshape(DH // 128, 128).T
    fc[:, 4:6] = b2.reshape(2, 128).T
    fc[:, 6] = np.asarray(bf1, np.float32).reshape(DH // 4)
    fc[:DOUT, 7] = np.asarray(bf2, np.float32).reshape(DOUT)
    shared = dict(
        w1dr=w1dr.astype(f8),
        w1bf=w1bf.astype(bf),
        wc_bf=np.ascontiguousarray(wc).astype(bf),
        fc32=fc,
    )
    return [dict(shared, **per_core[c]) for c in range(NCORES)]


def kernel(
    x, W1, b1, W2, b2, Wf1, bf1, Wf2, bf2, edge_index, batch, num_graphs, _trace=False
):
    assert int(num_graphs) == NG
    meta, per_core, cnt = _preprocess(
        np.asarray(x), np.asarray(edge_index), np.asarray(batch)
    )
    nc = _get_program(meta)
    in_maps = _make_in_maps(W1, b1, W2, b2, Wf1, bf1, Wf2, bf2, per_core, cnt, meta)
    res = bass_utils.run_bass_kernel_spmd(
        nc, in_maps, core_ids=list(range(NCORES)), trace=_trace
    )
    out = np.ascontiguousarray(np.asarray(res.results[0]["out"], np.float32).T)
    if _trace:
        kernel._last_results = res
    return out


# revision 54
# speedup vs baseline: 1.2283x; 1.2283x over previous
"""GCN classifier (2x GCNConv + mean-pool + 2-layer MLP) on 8 Trainium2 cores.

Sharding strategy (graph/data parallel per the hint):
- Nodes partitioned contiguously: core c owns dst nodes [c*6250, (c+1)*6250).
- conv1 (aggregate-then-transform): edges + self-loops partitioned by dst
  owner, grouped into 98 windows of 64 dst nodes, padded to 128-edge chunks
  (chunk counts maxed across cores -> one SPMD program). The host ships each
  core its incident edges' x rows pre-scaled by the full sym-norm
  dinv[src]*dinv[dst], quantized to fp8-e4m3, with the 64-wide 0/1 one-hot
  dst columns appended to each chunk (320 fp8 cols/chunk, one sequential
  DMA stream per batch of <=4 windows; no on-device one-hot build at all).
  The scatter-add runs on the PE: fp8 x chunk stationary (FWL fast weight
  load) x one-hot moving, accumulating in PSUM -> the aggregation lands
  feature-major, no transposes. Dense W1 applied with fp8 DoubleRow matmuls
  (both 128-row k-tiles in one pass, W1-pair stationary, agg streamed fp8)
  + bias + relu -> h1 kept feature-major in SBUF only (bf16). The DoubleRow
  weight loads hide under the previous group's conv2 matmuls (interleave).
- conv2 + mean-pool fused algebraically: with no nonlinearity between
  conv2's aggregation and the pooling, pooled sums satisfy
  pool[G] = sum_s A[s,G] * (h1[s] @ W2), where
  A[s,G] = dinv[s]*(sum_{e:src=s,dst in G} dinv[dst] + [batch[s]==G]*dinv[s])
  is built on host from edge_index/batch/deg only (structural data), with
  the mean-pool 1/cnt[G] (x64 for fp8 range) baked in. Each core computes
  p = h1 @ W2 (bf16) for its own node chunks and immediately accumulates
  pb^T @ A_chunk into persistent [128,64] PSUM tiles, keeping the pooled
  partials FEATURE-major -- the tail MLP then needs no transposes at all.
- Cross-core reduction as one 16KB fp8 AllGather + a pairwise DVE tree-sum
  (an AllGather is 2-3x cheaper than AllReduce on the CC cores), preceded
  by a dummy warm-up AllGather early in the program that absorbs the CC
  barrier + cold firmware setup under the main phase. /64 + bias + relu and
  the tiny MLP run replicated in feature-major layout (out lands as
  [DOUT, NG] directly); core 0's output wins.
- Pipelining: per-batch x_edges DMA (triple buffered, issue-ahead 2, first
  transfer split so the PE starts early), aggregation/dense/p-chunks
  interleaved batch by batch so the PE stays busy end to end.
"""

import sys
import types

import ml_dtypes
import numpy as np

try:
    import antenv  # noqa: F401

    if "antenv.axon_hooks" not in sys.modules:
        _m = types.ModuleType("antenv.axon_hooks")
        _m._hook = None
        _m.set_axon_ntff_profile_hook = lambda h: setattr(_m, "_hook", h)
        _m.get_axon_ntff_profile_hook = lambda: _m._hook
        sys.modules["antenv.axon_hooks"] = _m
except Exception:
    pass

import concourse.bacc as bacc
import concourse.mybir as mybir
import concourse.tile as tile
from concourse import bass_utils

F32 = mybir.dt.float32
BF16 = mybir.dt.bfloat16
F8 = mybir.dt.float8e4
AF = mybir.ActivationFunctionType
OP = mybir.AluOpType
DR = mybir.MatmulPerfMode.DoubleRow

N = 50000
E = 500000
DIN = 256
DH = 512
NG = 64
DOUT = 16

NCORES = 8
SLICE = N // NCORES  # 6250
WW = 64  # dst window width (one-hot width)
NW = (SLICE + WW - 1) // WW  # 98 windows
NPAD = 6272  # 49 * 128 node columns
NCHK = NPAD // 128  # 49 node chunks
NGRP = 13  # 12 groups of 512 node cols + 1 of 128

# tuning knobs
USE_DR_DENSE = True  # fp8 DoubleRow for the W1 dense
INTERLEAVE_PA = True  # emit pA of group g-1 between dense MMs of group g
CW = DIN + WW  # chunk width in the x_edges stream: 256 x cols + 64 one-hot

_COMPILED: dict = {}


def _group_info(g):
    """(first window, #windows, node col0, #node cols, first chunk, #chunks)"""
    if g < 12:
        return (8 * g, 8, 512 * g, 512, 4 * g, 4)
    return (96, 2, 6144, 128, 48, 1)


def _layout(K1):
    """Batches of <=4 windows: [(g, ws, {w: [(gcol, grel)]}, nch, c0)]."""
    batches = []
    gcol = 0
    for g in range(NGRP):
        w0, nwin, _, _, _, _ = _group_info(g)
        nhalf = 2 if nwin == 8 else 1
        for half in range(nhalf):
            ws = list(range(w0 + half * 4, min(w0 + (half + 1) * 4, w0 + nwin)))
            c0 = gcol
            rel = 0
            wch = {}
            for w in ws:
                lst = []
                for _ in range(int(K1[w])):
                    lst.append((gcol, rel))
                    gcol += 1
                    rel += 1
                wch[w] = lst
            batches.append((g, ws, wch, rel, c0))
    return batches, gcol


def _preprocess(x, edge_index, batch):
    src = np.asarray(edge_index[0], dtype=np.int64)
    dst = np.asarray(edge_index[1], dtype=np.int64)
    batch = np.asarray(batch, dtype=np.int64)

    deg = np.bincount(dst, minlength=N).astype(np.float64) + 1.0
    dinv = (1.0 / np.sqrt(deg)).astype(np.float32)
    cnt = np.maximum(np.bincount(batch, minlength=NG), 1)

    loops = np.arange(N, dtype=np.int64)

    # ---------- conv1: edges + self-loops grouped by (core, 64-window) ----------
    s1 = np.concatenate([src, loops])
    d1 = np.concatenate([dst, loops])
    norm1 = dinv[s1] * dinv[d1]
    core1 = d1 // SLICE
    win1 = (d1 % SLICE) // WW
    key1 = core1 * NW + win1
    order1 = np.argsort(key1, kind="stable")
    ss1, ds1, nn1 = s1[order1], d1[order1], norm1[order1]
    counts1 = np.bincount(key1, minlength=NCORES * NW).reshape(NCORES, NW)
    starts1 = np.zeros(NCORES * NW + 1, dtype=np.int64)
    np.cumsum(counts1.reshape(-1), out=starts1[1:])
    K1 = np.ceil(counts1.max(axis=0) / 128).astype(np.int64)  # [NW]

    meta = tuple(int(v) for v in K1)
    batches, C1 = _layout(K1)

    # ---------- fused conv2+pool coefficient matrix A[s, G] ----------
    gd = batch[dst]
    A = np.bincount(src * NG + gd, weights=dinv[dst].astype(np.float64),
                    minlength=N * NG).reshape(N, NG).astype(np.float32)
    A[loops, batch] += dinv
    A *= dinv[:, None]
    # bake the mean-pool 1/cnt into A, x64 to keep fp8 AR payloads in range;
    # the tail activation divides by 64 via its scale parameter
    A *= (64.0 / np.maximum(cnt, 1).astype(np.float32))[None, :]

    xf = np.asarray(x, np.float32)

    per_core = []
    for c in range(NCORES):
        src_cols = np.zeros((C1, 128), dtype=np.int64)
        nrm_cols = np.zeros((C1, 128), dtype=np.float32)
        dst_cols = np.full((C1, 128), -1.0, dtype=np.float32)
        for _g, ws, wch, _nch, _c0 in batches:
            for w in ws:
                gi = c * NW + w
                e0, e1 = starts1[gi], starts1[gi + 1]
                n_e = int(e1 - e0)
                cols = wch[w]
                k = len(cols)
                sv = np.zeros(k * 128, dtype=np.int64)
                sv[:n_e] = ss1[e0:e1]
                nv = np.zeros(k * 128, dtype=np.float32)
                nv[:n_e] = nn1[e0:e1]
                dv = np.full(k * 128, -1.0, dtype=np.float32)
                dv[:n_e] = (ds1[e0:e1] - (c * SLICE + w * WW)).astype(np.float32)
                for j, (gcol, _r) in enumerate(cols):
                    src_cols[gcol] = sv[j * 128 : (j + 1) * 128]
                    nrm_cols[gcol] = nv[j * 128 : (j + 1) * 128]
                    dst_cols[gcol] = dv[j * 128 : (j + 1) * 128]
        rows = xf[src_cols.reshape(-1)] * nrm_cols.reshape(-1)[:, None]
        combo = np.empty((C1, 128, CW), dtype=ml_dtypes.float8_e4m3)
        combo[:, :, :DIN] = rows.astype(ml_dtypes.float8_e4m3).reshape(C1, 128, DIN)
        combo[:, :, DIN:] = (
            dst_cols[:, :, None] == np.arange(WW, dtype=np.float32)[None, None, :]
        ).astype(ml_dtypes.float8_e4m3)
        x_edges = np.ascontiguousarray(combo.transpose(1, 0, 2)).reshape(128, C1 * CW)

        Ac = np.zeros((NPAD, NG), dtype=np.float32)
        Ac[:SLICE] = A[c * SLICE : (c + 1) * SLICE]
        a_sb = np.ascontiguousarray(
            Ac.reshape(NCHK, 128, NG).transpose(1, 0, 2)
        ).reshape(128, NCHK * NG).astype(ml_dtypes.bfloat16)

        per_core.append(dict(x_edges=x_edges, a_mat=a_sb))
    return meta, per_core, cnt.astype(np.float32)


def _build_program(meta):
    K1 = np.array(meta)
    batches, C1 = _layout(K1)
    max_nch = max(b[3] for b in batches)

    nc = bacc.Bacc("TRN2", target_bir_lowering=False, debug=False, num_devices=NCORES)

    def din(name, shape, dt=F32):
        return nc.dram_tensor(name, shape, dt, kind="ExternalInput").ap()

    x_edges = din("x_edges", [128, C1 * CW], F8)
    a_mat = din("a_mat", [128, NCHK * NG], BF16)
    w1dr = din("w1dr", [128, 2 * DH], F8)  # [p, k(2), m(4), 128] fp8 pairs
    w1bf = din("w1bf", [128, 2 * DH], BF16)  # [p, k(2), fo(512)] bf16 fallback
    # merged bf16 consts: [0:1024] w2b, [1024:1280] wf1, [1280:1296] wf2
    wc_bf = din("wc_bf", [128, 1296], BF16)
    # merged f32 consts: [0:4] b1c, [4:6] b2h, [6:7] bf1c, [7:8] bf2c (rows<16)
    fc32 = din("fc32", [128, 8])
    out = nc.dram_tensor("out", [DOUT, NG], F32, kind="ExternalOutput").ap()

    with tile.TileContext(nc) as tc:
        with (
            tc.tile_pool(name="const", bufs=1) as cp,
            tc.tile_pool(name="big", bufs=1) as bigp,
            tc.tile_pool(name="work", bufs=1) as wp,
            tc.tile_pool(name="psum", bufs=1, space="PSUM") as pp,
            tc.tile_pool(name="dram", bufs=1, space="DRAM") as dp,
        ):
            def load(ap_in, shape, dt=F32, pool=cp):
                t = pool.tile(shape, dt, name=ap_in.tensor.name + "_sb")
                nc.sync.dma_start(t[:], ap_in[:])
                return t

            # loads gating the pipeline start go first; the rest after batch 0

            h1s = [bigp.tile([128, NPAD], BF16, name=f"h1s_{k}") for k in range(4)]

            sfg_groups: dict = {}

            def sfg_of(g):
                # fp8 feature-major conv1 aggregation for group g: [p, k(2), n]
                if g not in sfg_groups:
                    sfg_groups[g] = wp.tile(
                        [128, 2, 512], F8, tag="sfg", bufs=2, name=f"sfg_{g}"
                    )
                return sfg_groups[g]

            # persistent feature-major pool partials: pgo[h] = [128 o, 64 G]
            pgo = [pp.tile([128, NG], F32, name=f"pgo_{h}") for h in range(2)]
            g_local = dp.tile([128, 2 * NG], F8, name="gl")
            g_ag = dp.tile([NCORES * 128, 2 * NG], F8, addr_space="Shared", name="gag")
            gs_all = wp.tile([128, NCORES, 2 * NG], F8, name="gs_all")
            red = wp.tile([128, 2 * NG], F32, name="red")

            def emit_allgather():
                # AllGather the 8 fp8 pool partials (cheaper than AllReduce:
                # no reduce phase on the CC cores) and sum them on the DVE.
                gsb = wp.tile([128, 2, NG], F8, name="gsb")
                for h in range(2):
                    nc.vector.tensor_copy(gsb[:, h, :], pgo[h][:])
                nc.sync.dma_start(g_local[:], gsb[:].rearrange("p h g -> p (h g)"))
                nc.gpsimd.collective_compute(
                    "AllGather",
                    OP.bypass,
                    replica_groups=[list(range(NCORES))],
                    ins=[g_local.opt()],
                    outs=[g_ag.opt()],
                )
                nc.sync.dma_start(
                    gs_all[:],
                    g_ag[:].rearrange("(r p) c -> p r c", r=NCORES),
                )
                # pairwise tree-sum of the 8 partials (contiguous adds beat a
                # strided tensor_reduce on the DVE)
                t4 = wp.tile([128, 4, 2 * NG], F32, name="red4")
                nc.vector.tensor_tensor(
                    out=t4[:], in0=gs_all[:, 0:4, :], in1=gs_all[:, 4:8, :], op=OP.add
                )
                t2 = wp.tile([128, 2, 2 * NG], F32, name="red2")
                nc.vector.tensor_tensor(
                    out=t2[:], in0=t4[:, 0:2, :], in1=t4[:, 2:4, :], op=OP.add
                )
                nc.vector.tensor_tensor(
                    out=red[:], in0=t2[:, 0, :], in1=t2[:, 1, :], op=OP.add
                )

            def emit_stream(bi):
                """One G1 DMA covering one batch; one-hot cols ride along."""
                _g, _ws, _wch, nch, c0 = batches[bi]
                G1 = wp.tile([128, nch, CW], F8, tag="G1", bufs=3, name=f"g1b_{bi}")
                if bi == 0:
                    # split the first transfer so the opening windows land
                    # (and the PE starts) sooner
                    n1 = max(1, nch // 2)
                    nc.sync.dma_start(
                        G1[:, :n1, :].rearrange("p c d -> p (c d)"),
                        x_edges[:, c0 * CW : (c0 + n1) * CW],
                    )
                    nc.sync.dma_start(
                        G1[:, n1:, :].rearrange("p c d -> p (c d)"),
                        x_edges[:, (c0 + n1) * CW : (c0 + nch) * CW],
                    )
                else:
                    nc.sync.dma_start(
                        G1[:].rearrange("p c d -> p (c d)"),
                        x_edges[:, c0 * CW : (c0 + nch) * CW],
                    )
                return G1

            def emit_batch(g, ws, wch, nch, c0, G1):
                sfg = sfg_of(g)
                nw = len(ws)
                wb0 = ws[0] - 8 * g
                pa = pp.tile([128, nw, 2, WW], F32, tag="agg", bufs=2, name=f"pa_{ws[0]}")
                for w in ws:
                    cols = wch[w]
                    wrel = w - ws[0]
                    for j, (_gcol, grel) in enumerate(cols):
                        for h in range(2):
                            nc.tensor.matmul(
                                out=pa[:, wrel, h, :],
                                lhsT=G1[:, grel, h * 128 : (h + 1) * 128],
                                rhs=G1[:, grel, DIN:CW],
                                start=(j == 0),
                                stop=(j == len(cols) - 1),
                            )
                for h in range(2):
                    nc.vector.tensor_copy(
                        sfg[:, h, wb0 * WW : (wb0 + nw) * WW],
                        pa[:, :, h, :],
                    )

            def emit_dense_mm(g, m):
                _, _, n0, ncols, _, _ = _group_info(g)
                sfg = sfg_of(g)
                ph = pp.tile([128, 512], F32, tag="h1", bufs=2, name=f"ph_{g}_{m}")
                if USE_DR_DENSE:
                    nc.tensor.matmul(
                        out=ph[:, :ncols],
                        lhsT=w1_sb[:, :, m, :],
                        rhs=sfg[:, :, :ncols],
                        start=True,
                        stop=True,
                        perf_mode=DR,
                    )
                else:
                    for k in range(2):
                        nc.tensor.matmul(
                            out=ph[:, :ncols],
                            lhsT=w1f_sb[:, k, m * 128 : (m + 1) * 128],
                            rhs=sfg[:, k, :ncols],
                            start=(k == 0),
                            stop=(k == 1),
                        )
                nc.scalar.activation(
                    h1s[m][:, n0 : n0 + ncols], ph[:, :ncols], AF.Relu,
                    bias=fc_sb[:, m : m + 1],
                )

            def emit_pA(cc):
                c0 = cc * 128
                ppm = pp.tile([128, DH // 2], F32, tag="p2", bufs=2, name=f"ppm_{cc}")
                for k in range(4):
                    nc.tensor.matmul(
                        out=ppm[:],
                        lhsT=h1s[k][:, c0 : c0 + 128],
                        rhs=wc_sb[:, k * (DH // 2) : (k + 1) * (DH // 2)],
                        start=(k == 0),
                        stop=(k == 3),
                    )
                pb = wp.tile([128, DH // 2], BF16, tag="pb", bufs=2, name=f"pb_{cc}")
                nc.vector.tensor_copy(pb[:], ppm[:])
                for h in range(2):
                    nc.tensor.matmul(
                        out=pgo[h][:],
                        lhsT=pb[:, h * 128 : (h + 1) * 128],
                        rhs=a_sb[:, cc * NG : (cc + 1) * NG],
                        start=(cc == 0),
                        stop=(cc == NCHK - 1),
                    )

            streams = {0: emit_stream(0), 1: emit_stream(1)}
            if USE_DR_DENSE:
                w1_sb = load(w1dr, [128, 2, 4, 128], F8)
            else:
                w1f_sb = load(w1bf, [128, 2, DH], BF16)
            fc_sb = load(fc32, [128, 8])
            # warm-up collective: absorbs the CC barrier + cold firmware setup
            # during the main phase so the real AllGather launches warm. Kept
            # to a single probe so a slow barrier can't push the chain past
            # the end of the main phase.
            warm_l = dp.tile([128, 128], F8, name="warm_l")
            warm_ag = dp.tile([128 * 8, 128], F8, addr_space="Shared", name="warm_ag")
            nc.gpsimd.collective_compute(
                "AllGather", OP.bypass, replica_groups=[list(range(NCORES))],
                ins=[warm_l.opt()], outs=[warm_ag.opt()],
            )
            bidx = 0
            pending = []
            for g in range(NGRP):
                _, nwin, _, _, cc0, nccs = _group_info(g)
                nb = 2 if nwin == 8 else 1
                for _b in range(nb):
                    if bidx + 2 < len(batches):
                        streams[bidx + 2] = emit_stream(bidx + 2)
                    G1 = streams.pop(bidx)
                    emit_batch(*batches[bidx], G1)
                    bidx += 1
                    if bidx == 1:
                        a_sb = load(a_mat, [128, NCHK * NG], BF16)
                        wc_sb = load(wc_bf, [128, 1296], BF16)
                for m in range(4):
                    emit_dense_mm(g, m)
                    if INTERLEAVE_PA and pending:
                        emit_pA(pending.pop(0))
                if INTERLEAVE_PA:
                    pending.extend(range(cc0, cc0 + nccs))
                else:
                    for cc in range(cc0, cc0 + nccs):
                        emit_pA(cc)
            for cc in pending:
                emit_pA(cc)

            # ---- tail: AllGather + DVE reduce + mean/bias/relu + MLP,
            # all feature-major (1/cnt baked into A on host, x64; /64 here)
            emit_allgather()
            curv = red[:].rearrange("p (h g) -> p h g", h=2)
            grelu = wp.tile([128, 2, NG], BF16, name="grelu")
            for h in range(2):
                nc.scalar.activation(
                    grelu[:, h, :], curv[:, h, :], AF.Relu,
                    bias=fc_sb[:, 4 + h : 5 + h], scale=1.0 / 64.0,
                )
            pz = pp.tile([128, NG], F32, tag="p2", bufs=2, name="pz")
            for k in range(2):
                nc.tensor.matmul(
                    out=pz[:],
                    lhsT=wc_sb[:, 1024 + k * 128 : 1024 + (k + 1) * 128],
                    rhs=grelu[:, k, :],
                    start=(k == 0),
                    stop=(k == 1),
                )
            zsb = wp.tile([128, NG], BF16, name="zsb")
            nc.scalar.activation(zsb[:], pz[:], AF.Relu, bias=fc_sb[:, 6:7])
            po = pp.tile([DOUT, NG], F32, tag="agg", bufs=2, name="po")
            nc.tensor.matmul(
                out=po[:], lhsT=wc_sb[:, 1280:1296], rhs=zsb[:], start=True, stop=True
            )
            osb = wp.tile([DOUT, NG], F32, name="osb")
            nc.scalar.activation(osb[:], po[:], AF.Relu, bias=fc_sb[:16, 7:8])
            nc.sync.dma_start(out[:], osb[:])

    nc.compile()
    return nc


def _get_program(meta):
    if meta not in _COMPILED:
        _COMPILED[meta] = _build_program(meta)
    return _COMPILED[meta]


def _make_in_maps(W1, b1, W2, b2, Wf1, bf1, Wf2, bf2, per_core, cnt, meta):
    bf = ml_dtypes.bfloat16
    f8 = ml_dtypes.float8_e4m3
    W1 = np.asarray(W1, np.float32)
    W2 = np.asarray(W2, np.float32)
    Wf1 = np.asarray(Wf1, np.float32)
    b2 = np.asarray(b2, np.float32)
    K1 = np.array(meta)
    batches, _C1 = _layout(K1)
    max_nch = max(b[3] for b in batches)

    # w1dr[p, k, m, c] = W1[k*128+p, m*128+c]
    w1dr = np.ascontiguousarray(
        W1.reshape(2, 128, 4, 128).transpose(1, 0, 2, 3).reshape(128, 2 * DH)
    )
    w1bf = np.ascontiguousarray(
        W1.reshape(2, 128, DH).transpose(1, 0, 2).reshape(128, 2 * DH)
    )
    w2b = np.ascontiguousarray(
        np.concatenate([W2[k * 128 : (k + 1) * 128, :] for k in range(4)], axis=1)
    )
    wf1b = np.ascontiguousarray(
        Wf1.reshape(2, 128, DH // 4).transpose(1, 0, 2).reshape(128, 2 * (DH // 4))
    )
    wc = np.concatenate([w2b, wf1b, np.asarray(Wf2, np.float32)], axis=1)
    fc = np.zeros((128, 8), np.float32)
    fc[:, 0:4] = np.asarray(b1, np.float32).reshape(DH // 128, 128).T
    fc[:, 4:6] = b2.reshape(2, 128).T
    fc[:, 6] = np.asarray(bf1, np.float32).reshape(DH // 4)
    fc[:DOUT, 7] = np.asarray(bf2, np.float32).reshape(DOUT)
    shared = dict(
        w1dr=w1dr.astype(f8),
        w1bf=w1bf.astype(bf),
        wc_bf=np.ascontiguousarray(wc).astype(bf),
        fc32=fc,
    )
    return [dict(shared, **per_core[c]) for c in range(NCORES)]


def kernel(
    x, W1, b1, W2, b2, Wf1, bf1, Wf2, bf2, edge_index, batch, num_graphs, _trace=False
):
    assert int(num_graphs) == NG
    meta, per_core, cnt = _preprocess(
        np.asarray(x), np.asarray(edge_index), np.asarray(batch)
    )
    nc = _get_program(meta)
    in_maps = _make_in_maps(W1, b1, W2, b2, Wf1, bf1, Wf2, bf2, per_core, cnt, meta)
    res = bass_utils.run_bass_kernel_spmd(
        nc, in_maps, core_ids=list(range(NCORES)), trace=_trace
    )
    out = np.ascontiguousarray(np.asarray(res.results[0]["out"], np.float32).T)
    if _trace:
        kernel._last_results = res
    return out


# revision 59
# speedup vs baseline: 1.2485x; 1.0165x over previous
"""GCN classifier (2x GCNConv + mean-pool + 2-layer MLP) on 8 Trainium2 cores.

Sharding strategy (graph/data parallel per the hint):
- Nodes partitioned contiguously: core c owns dst nodes [c*6250, (c+1)*6250).
- conv1 (aggregate-then-transform): edges + self-loops partitioned by dst
  owner, grouped into 98 windows of 64 dst nodes, padded to 128-edge chunks
  (chunk counts maxed across cores -> one SPMD program). The host ships each
  core its incident edges' x rows pre-scaled by the full sym-norm
  dinv[src]*dinv[dst], quantized to fp8-e4m3, with the 64-wide 0/1 one-hot
  dst columns appended to each chunk (320 fp8 cols/chunk, one sequential
  DMA stream per batch of <=4 windows; no on-device one-hot build at all).
  The scatter-add runs on the PE: fp8 x chunk stationary (FWL fast weight
  load) x one-hot moving, accumulating in PSUM -> the aggregation lands
  feature-major, no transposes. Dense W1 applied with fp8 DoubleRow matmuls
  (both 128-row k-tiles in one pass, W1-pair stationary, agg streamed fp8)
  + bias + relu -> h1 kept feature-major in SBUF only (bf16). The DoubleRow
  weight loads hide under the previous group's conv2 matmuls (interleave).
- conv2 + mean-pool fused algebraically: with no nonlinearity between
  conv2's aggregation and the pooling, pooled sums satisfy
  pool[G] = sum_s A[s,G] * (h1[s] @ W2), where
  A[s,G] = dinv[s]*(sum_{e:src=s,dst in G} dinv[dst] + [batch[s]==G]*dinv[s])
  is built on host from edge_index/batch/deg only (structural data), with
  the mean-pool 1/cnt[G] (x64 for fp8 range) baked in. Each core computes
  p = h1 @ W2 (bf16) for its own node chunks and immediately accumulates
  pb^T @ A_chunk into persistent [128,64] PSUM tiles, keeping the pooled
  partials FEATURE-major -- the tail MLP then needs no transposes at all.
- Cross-core reduction as one 16KB fp8 AllGather + a pairwise DVE tree-sum
  (an AllGather is 2-3x cheaper than AllReduce on the CC cores), preceded
  by a dummy warm-up AllGather early in the program that absorbs the CC
  barrier + cold firmware setup under the main phase. /64 + bias + relu and
  the tiny MLP run replicated in feature-major layout (out lands as
  [DOUT, NG] directly); core 0's output wins.
- Pipelining: per-batch x_edges DMA (triple buffered, issue-ahead 2, first
  transfer split so the PE starts early), aggregation/dense/p-chunks
  interleaved batch by batch so the PE stays busy end to end.
"""

import sys
import types

import ml_dtypes
import numpy as np

try:
    import antenv  # noqa: F401

    if "antenv.axon_hooks" not in sys.modules:
        _m = types.ModuleType("antenv.axon_hooks")
        _m._hook = None
        _m.set_axon_ntff_profile_hook = lambda h: setattr(_m, "_hook", h)
        _m.get_axon_ntff_profile_hook = lambda: _m._hook
        sys.modules["antenv.axon_hooks"] = _m
except Exception:
    pass

import concourse.bacc as bacc
import concourse.mybir as mybir
import concourse.tile as tile
from concourse import bass_utils

F32 = mybir.dt.float32
BF16 = mybir.dt.bfloat16
F8 = mybir.dt.float8e4
AF = mybir.ActivationFunctionType
OP = mybir.AluOpType
DR = mybir.MatmulPerfMode.DoubleRow

N = 50000
E = 500000
DIN = 256
DH = 512
NG = 64
DOUT = 16

NCORES = 8
SLICE = N // NCORES  # 6250
WW = 64  # dst window width (one-hot width)
NW = (SLICE + WW - 1) // WW  # 98 windows
NPAD = 6272  # 49 * 128 node columns
NCHK = NPAD // 128  # 49 node chunks
NGRP = 13  # 12 groups of 512 node cols + 1 of 128

# tuning knobs
USE_DR_DENSE = True  # fp8 DoubleRow for the W1 dense
INTERLEAVE_PA = True  # emit pA of group g-1 between dense MMs of group g
CW = DIN + WW  # chunk width in the x_edges stream: 256 x cols + 64 one-hot

_COMPILED: dict = {}


def _group_info(g):
    """(first window, #windows, node col0, #node cols, first chunk, #chunks)"""
    if g < 12:
        return (8 * g, 8, 512 * g, 512, 4 * g, 4)
    return (96, 2, 6144, 128, 48, 1)


def _layout(K1):
    """Batches of <=4 windows: [(g, ws, {w: [(gcol, grel)]}, nch, c0)]."""
    batches = []
    gcol = 0
    for g in range(NGRP):
        w0, nwin, _, _, _, _ = _group_info(g)
        nhalf = 2 if nwin == 8 else 1
        for half in range(nhalf):
            ws = list(range(w0 + half * 4, min(w0 + (half + 1) * 4, w0 + nwin)))
            c0 = gcol
            rel = 0
            wch = {}
            for w in ws:
                lst = []
                for _ in range(int(K1[w])):
                    lst.append((gcol, rel))
                    gcol += 1
                    rel += 1
                wch[w] = lst
            batches.append((g, ws, wch, rel, c0))
    return batches, gcol


def _preprocess(x, edge_index, batch):
    src = np.asarray(edge_index[0], dtype=np.int64)
    dst = np.asarray(edge_index[1], dtype=np.int64)
    batch = np.asarray(batch, dtype=np.int64)

    deg = np.bincount(dst, minlength=N).astype(np.float64) + 1.0
    dinv = (1.0 / np.sqrt(deg)).astype(np.float32)
    cnt = np.maximum(np.bincount(batch, minlength=NG), 1)

    loops = np.arange(N, dtype=np.int64)

    # ---------- conv1: edges + self-loops grouped by (core, 64-window) ----------
    s1 = np.concatenate([src, loops])
    d1 = np.concatenate([dst, loops])
    norm1 = dinv[s1] * dinv[d1]
    core1 = d1 // SLICE
    win1 = (d1 % SLICE) // WW
    key1 = core1 * NW + win1
    order1 = np.argsort(key1, kind="stable")
    ss1, ds1, nn1 = s1[order1], d1[order1], norm1[order1]
    counts1 = np.bincount(key1, minlength=NCORES * NW).reshape(NCORES, NW)
    starts1 = np.zeros(NCORES * NW + 1, dtype=np.int64)
    np.cumsum(counts1.reshape(-1), out=starts1[1:])
    K1 = np.ceil(counts1.max(axis=0) / 128).astype(np.int64)  # [NW]

    meta = tuple(int(v) for v in K1)
    batches, C1 = _layout(K1)

    # ---------- fused conv2+pool coefficient matrix A[s, G] ----------
    gd = batch[dst]
    A = np.bincount(src * NG + gd, weights=dinv[dst].astype(np.float64),
                    minlength=N * NG).reshape(N, NG).astype(np.float32)
    A[loops, batch] += dinv
    A *= dinv[:, None]
    # bake the mean-pool 1/cnt into A, x64 to keep fp8 AR payloads in range;
    # the tail activation divides by 64 via its scale parameter
    A *= (64.0 / np.maximum(cnt, 1).astype(np.float32))[None, :]

    xf = np.asarray(x, np.float32)

    per_core = []
    for c in range(NCORES):
        src_cols = np.zeros((C1, 128), dtype=np.int64)
        nrm_cols = np.zeros((C1, 128), dtype=np.float32)
        dst_cols = np.full((C1, 128), -1.0, dtype=np.float32)
        for _g, ws, wch, _nch, _c0 in batches:
            for w in ws:
                gi = c * NW + w
                e0, e1 = starts1[gi], starts1[gi + 1]
                n_e = int(e1 - e0)
                cols = wch[w]
                k = len(cols)
                sv = np.zeros(k * 128, dtype=np.int64)
                sv[:n_e] = ss1[e0:e1]
                nv = np.zeros(k * 128, dtype=np.float32)
                nv[:n_e] = nn1[e0:e1]
                dv = np.full(k * 128, -1.0, dtype=np.float32)
                dv[:n_e] = (ds1[e0:e1] - (c * SLICE + w * WW)).astype(np.float32)
                for j, (gcol, _r) in enumerate(cols):
                    src_cols[gcol] = sv[j * 128 : (j + 1) * 128]
                    nrm_cols[gcol] = nv[j * 128 : (j + 1) * 128]
                    dst_cols[gcol] = dv[j * 128 : (j + 1) * 128]
        rows = xf[src_cols.reshape(-1)] * nrm_cols.reshape(-1)[:, None]
        combo = np.empty((C1, 128, CW), dtype=ml_dtypes.float8_e4m3)
        combo[:, :, :DIN] = rows.astype(ml_dtypes.float8_e4m3).reshape(C1, 128, DIN)
        combo[:, :, DIN:] = (
            dst_cols[:, :, None] == np.arange(WW, dtype=np.float32)[None, None, :]
        ).astype(ml_dtypes.float8_e4m3)
        x_edges = np.ascontiguousarray(combo.transpose(1, 0, 2)).reshape(128, C1 * CW)

        Ac = np.zeros((NPAD, NG), dtype=np.float32)
        Ac[:SLICE] = A[c * SLICE : (c + 1) * SLICE]
        a_sb = np.ascontiguousarray(
            Ac.reshape(NCHK, 128, NG).transpose(1, 0, 2)
        ).reshape(128, NCHK * NG).astype(ml_dtypes.bfloat16)

        per_core.append(dict(x_edges=x_edges, a_mat=a_sb))
    return meta, per_core, cnt.astype(np.float32)


def _build_program(meta):
    K1 = np.array(meta)
    batches, C1 = _layout(K1)
    max_nch = max(b[3] for b in batches)

    nc = bacc.Bacc("TRN2", target_bir_lowering=False, debug=False, num_devices=NCORES)

    def din(name, shape, dt=F32):
        return nc.dram_tensor(name, shape, dt, kind="ExternalInput").ap()

    x_edges = din("x_edges", [128, C1 * CW], F8)
    a_mat = din("a_mat", [128, NCHK * NG], BF16)
    w1dr = din("w1dr", [128, 2 * DH], F8)  # [p, k(2), m(4), 128] fp8 pairs
    w1bf = din("w1bf", [128, 2 * DH], BF16)  # [p, k(2), fo(512)] bf16 fallback
    # merged bf16 consts: [0:1024] w2b, [1024:1280] wf1, [1280:1296] wf2
    wc_bf = din("wc_bf", [128, 1296], BF16)
    # merged f32 consts: [0:4] b1c, [4:6] b2h, [6:7] bf1c, [7:8] bf2c (rows<16)
    fc32 = din("fc32", [128, 8])
    out = nc.dram_tensor("out", [DOUT, NG], F32, kind="ExternalOutput").ap()

    with tile.TileContext(nc) as tc:
        with (
            tc.tile_pool(name="const", bufs=1) as cp,
            tc.tile_pool(name="big", bufs=1) as bigp,
            tc.tile_pool(name="work", bufs=1) as wp,
            tc.tile_pool(name="psum", bufs=1, space="PSUM") as pp,
            tc.tile_pool(name="dram", bufs=1, space="DRAM") as dp,
        ):
            def load(ap_in, shape, dt=F32, pool=cp):
                t = pool.tile(shape, dt, name=ap_in.tensor.name + "_sb")
                nc.sync.dma_start(t[:], ap_in[:])
                return t

            # loads gating the pipeline start go first; the rest after batch 0

            h1s = [bigp.tile([128, NPAD], BF16, name=f"h1s_{k}") for k in range(4)]

            sfg_groups: dict = {}

            def sfg_of(g):
                # fp8 feature-major conv1 aggregation for group g: [p, k(2), n]
                if g not in sfg_groups:
                    sfg_groups[g] = wp.tile(
                        [128, 2, 512], F8, tag="sfg", bufs=2, name=f"sfg_{g}"
                    )
                return sfg_groups[g]

            # persistent feature-major pool partials: pgo[h] = [128 o, 64 G]
            pgo = [pp.tile([128, NG], F32, name=f"pgo_{h}") for h in range(2)]
            g_local = dp.tile([128, 2 * NG], F8, name="gl")
            g_ag = dp.tile([NCORES * 128, 2 * NG], F8, addr_space="Shared", name="gag")
            gs_all = wp.tile([128, NCORES, 2 * NG], F8, name="gs_all")
            red = wp.tile([128, 2 * NG], F32, name="red")

            def emit_allgather():
                # AllGather the 8 fp8 pool partials (cheaper than AllReduce:
                # no reduce phase on the CC cores) and sum them on the DVE.
                gsb = wp.tile([128, 2, NG], F8, name="gsb")
                for h in range(2):
                    nc.vector.tensor_copy(gsb[:, h, :], pgo[h][:])
                nc.sync.dma_start(g_local[:], gsb[:].rearrange("p h g -> p (h g)"))
                nc.gpsimd.collective_compute(
                    "AllGather",
                    OP.bypass,
                    replica_groups=[list(range(NCORES))],
                    ins=[g_local.opt()],
                    outs=[g_ag.opt()],
                )
                nc.sync.dma_start(
                    gs_all[:],
                    g_ag[:].rearrange("(r p) c -> p r c", r=NCORES),
                )
                # pairwise tree-sum of the 8 partials (contiguous adds beat a
                # strided tensor_reduce on the DVE)
                t4 = wp.tile([128, 4, 2 * NG], F32, name="red4")
                nc.vector.tensor_tensor(
                    out=t4[:], in0=gs_all[:, 0:4, :], in1=gs_all[:, 4:8, :], op=OP.add
                )
                t2 = wp.tile([128, 2, 2 * NG], F32, name="red2")
                nc.vector.tensor_tensor(
                    out=t2[:], in0=t4[:, 0:2, :], in1=t4[:, 2:4, :], op=OP.add
                )
                nc.vector.tensor_tensor(
                    out=red[:], in0=t2[:, 0, :], in1=t2[:, 1, :], op=OP.add
                )

            def emit_stream(bi):
                """One G1 DMA covering one batch; one-hot cols ride along."""
                _g, _ws, _wch, nch, c0 = batches[bi]
                G1 = wp.tile([128, nch, CW], F8, tag="G1", bufs=4, name=f"g1b_{bi}")
                if bi == 0:
                    # split the first transfer so the opening windows land
                    # (and the PE starts) sooner
                    n1 = max(1, nch // 2)
                    nc.sync.dma_start(
                        G1[:, :n1, :].rearrange("p c d -> p (c d)"),
                        x_edges[:, c0 * CW : (c0 + n1) * CW],
                    )
                    nc.sync.dma_start(
                        G1[:, n1:, :].rearrange("p c d -> p (c d)"),
                        x_edges[:, (c0 + n1) * CW : (c0 + nch) * CW],
                    )
                else:
                    nc.sync.dma_start(
                        G1[:].rearrange("p c d -> p (c d)"),
                        x_edges[:, c0 * CW : (c0 + nch) * CW],
                    )
                return G1

            def emit_batch(g, ws, wch, nch, c0, G1):
                sfg = sfg_of(g)
                nw = len(ws)
                wb0 = ws[0] - 8 * g
                pa = pp.tile([128, nw, 2, WW], F32, tag="agg", bufs=2, name=f"pa_{ws[0]}")
                for w in ws:
                    cols = wch[w]
                    wrel = w - ws[0]
                    for j, (_gcol, grel) in enumerate(cols):
                        for h in range(2):
                            nc.tensor.matmul(
                                out=pa[:, wrel, h, :],
                                lhsT=G1[:, grel, h * 128 : (h + 1) * 128],
                                rhs=G1[:, grel, DIN:CW],
                                start=(j == 0),
                                stop=(j == len(cols) - 1),
                            )
                for h in range(2):
                    nc.vector.tensor_copy(
                        sfg[:, h, wb0 * WW : (wb0 + nw) * WW],
                        pa[:, :, h, :],
                    )

            def emit_dense_mm(g, m):
                _, _, n0, ncols, _, _ = _group_info(g)
                sfg = sfg_of(g)
                ph = pp.tile([128, 512], F32, tag="h1", bufs=2, name=f"ph_{g}_{m}")
                if USE_DR_DENSE:
                    nc.tensor.matmul(
                        out=ph[:, :ncols],
                        lhsT=w1_sb[:, :, m, :],
                        rhs=sfg[:, :, :ncols],
                        start=True,
                        stop=True,
                        perf_mode=DR,
                    )
                else:
                    for k in range(2):
                        nc.tensor.matmul(
                            out=ph[:, :ncols],
                            lhsT=w1f_sb[:, k, m * 128 : (m + 1) * 128],
                            rhs=sfg[:, k, :ncols],
                            start=(k == 0),
                            stop=(k == 1),
                        )
                nc.scalar.activation(
                    h1s[m][:, n0 : n0 + ncols], ph[:, :ncols], AF.Relu,
                    bias=fc_sb[:, m : m + 1],
                )

            pb_tiles = {}
            a_pending = []

            def emit_p(cc):
                c0 = cc * 128
                ppm = pp.tile([128, DH // 2], F32, tag="p2", bufs=2, name=f"ppm_{cc}")
                for k in range(4):
                    nc.tensor.matmul(
                        out=ppm[:],
                        lhsT=h1s[k][:, c0 : c0 + 128],
                        rhs=wc_sb[:, k * (DH // 2) : (k + 1) * (DH // 2)],
                        start=(k == 0),
                        stop=(k == 3),
                    )
                pb = wp.tile([128, DH // 2], BF16, tag="pb", bufs=2, name=f"pb_{cc}")
                nc.vector.tensor_copy(pb[:], ppm[:])
                pb_tiles[cc] = pb

            def emit_A(cc):
                pb = pb_tiles.pop(cc)
                for h in range(2):
                    nc.tensor.matmul(
                        out=pgo[h][:],
                        lhsT=pb[:, h * 128 : (h + 1) * 128],
                        rhs=a_sb[:, cc * NG : (cc + 1) * NG],
                        start=(cc == 0),
                        stop=(cc == NCHK - 1),
                    )

            def emit_pA(cc):
                # software-pipelined by one chunk: the A-matmuls of chunk c
                # are emitted after the p-matmuls of chunk c+1, so their pb
                # stationary (a DVE cast of this chunk's PSUM) has landed and
                # the weight load is off the critical path
                emit_p(cc)
                if a_pending:
                    emit_A(a_pending.pop(0))
                a_pending.append(cc)

            streams = {0: emit_stream(0), 1: emit_stream(1), 2: emit_stream(2)}
            if USE_DR_DENSE:
                w1_sb = load(w1dr, [128, 2, 4, 128], F8)
            else:
                w1f_sb = load(w1bf, [128, 2, DH], BF16)
            fc_sb = load(fc32, [128, 8])
            # warm-up collective: absorbs the CC barrier + cold firmware setup
            # during the main phase so the real AllGather launches warm. Kept
            # to a single probe so a slow barrier can't push the chain past
            # the end of the main phase.
            warm_l = dp.tile([128, 128], F8, name="warm_l")
            warm_ag = dp.tile([128 * 8, 128], F8, addr_space="Shared", name="warm_ag")
            nc.gpsimd.collective_compute(
                "AllGather", OP.bypass, replica_groups=[list(range(NCORES))],
                ins=[warm_l.opt()], outs=[warm_ag.opt()],
            )
            bidx = 0
            pending = []
            for g in range(NGRP):
                _, nwin, _, _, cc0, nccs = _group_info(g)
                nb = 2 if nwin == 8 else 1
                for _b in range(nb):
                    if bidx + 3 < len(batches) and bidx + 3 not in streams:
                        streams[bidx + 3] = emit_stream(bidx + 3)
                    G1 = streams.pop(bidx)
                    emit_batch(*batches[bidx], G1)
                    bidx += 1
                    if bidx == 1:
                        a_sb = load(a_mat, [128, NCHK * NG], BF16)
                        wc_sb = load(wc_bf, [128, 1296], BF16)
                for m in range(4):
                    emit_dense_mm(g, m)
                    if INTERLEAVE_PA and pending:
                        emit_pA(pending.pop(0))
                if INTERLEAVE_PA:
                    pending.extend(range(cc0, cc0 + nccs))
                else:
                    for cc in range(cc0, cc0 + nccs):
                        emit_pA(cc)
            for cc in pending:
                emit_pA(cc)
            for cc in a_pending:
                emit_A(cc)

            # ---- tail: AllGather + DVE reduce + mean/bias/relu + MLP,
            # all feature-major (1/cnt baked into A on host, x64; /64 here)
            emit_allgather()
            curv = red[:].rearrange("p (h g) -> p h g", h=2)
            grelu = wp.tile([128, 2, NG], BF16, name="grelu")
            for h in range(2):
                nc.scalar.activation(
                    grelu[:, h, :], curv[:, h, :], AF.Relu,
                    bias=fc_sb[:, 4 + h : 5 + h], scale=1.0 / 64.0,
                )
            pz = pp.tile([128, NG], F32, tag="p2", bufs=2, name="pz")
            for k in range(2):
                nc.tensor.matmul(
                    out=pz[:],
                    lhsT=wc_sb[:, 1024 + k * 128 : 1024 + (k + 1) * 128],
                    rhs=grelu[:, k, :],
                    start=(k == 0),
                    stop=(k == 1),
                )
            zsb = wp.tile([128, NG], BF16, name="zsb")
            nc.scalar.activation(zsb[:], pz[:], AF.Relu, bias=fc_sb[:, 6:7])
            po = pp.tile([DOUT, NG], F32, tag="agg", bufs=2, name="po")
            nc.tensor.matmul(
                out=po[:], lhsT=wc_sb[:, 1280:1296], rhs=zsb[:], start=True, stop=True
            )
            osb = wp.tile([DOUT, NG], F32, name="osb")
            nc.scalar.activation(osb[:], po[:], AF.Relu, bias=fc_sb[:16, 7:8])
            nc.sync.dma_start(out[:], osb[:])

    nc.compile()
    return nc


def _get_program(meta):
    if meta not in _COMPILED:
        _COMPILED[meta] = _build_program(meta)
    return _COMPILED[meta]


def _make_in_maps(W1, b1, W2, b2, Wf1, bf1, Wf2, bf2, per_core, cnt, meta):
    bf = ml_dtypes.bfloat16
    f8 = ml_dtypes.float8_e4m3
    W1 = np.asarray(W1, np.float32)
    W2 = np.asarray(W2, np.float32)
    Wf1 = np.asarray(Wf1, np.float32)
    b2 = np.asarray(b2, np.float32)
    K1 = np.array(meta)
    batches, _C1 = _layout(K1)
    max_nch = max(b[3] for b in batches)

    # w1dr[p, k, m, c] = W1[k*128+p, m*128+c]
    w1dr = np.ascontiguousarray(
        W1.reshape(2, 128, 4, 128).transpose(1, 0, 2, 3).reshape(128, 2 * DH)
    )
    w1bf = np.ascontiguousarray(
        W1.reshape(2, 128, DH).transpose(1, 0, 2).reshape(128, 2 * DH)
    )
    w2b = np.ascontiguousarray(
        np.concatenate([W2[k * 128 : (k + 1) * 128, :] for k in range(4)], axis=1)
    )
    wf1b = np.ascontiguousarray(
        Wf1.reshape(2, 128, DH // 4).transpose(1, 0, 2).reshape(128, 2 * (DH // 4))
    )
    wc = np.concatenate([w2b, wf1b, np.asarray(Wf2, np.float32)], axis=1)
    fc = np.zeros((128, 8), np.float32)
    fc[:, 0:4] = np.asarray(b1, np.float32).reshape(DH // 128, 128).T
    fc[:, 4:6] = b2.reshape(2, 128).T
    fc[:, 6] = np.asarray(bf1, np.float32).reshape(DH // 4)
    fc[:DOUT, 7] = np.asarray(bf2, np.float32).reshape(DOUT)
    shared = dict(
        w1dr=w1dr.astype(f8),
        w1bf=w1bf.astype(bf),
        wc_bf=np.ascontiguousarray(wc).astype(bf),
        fc32=fc,
    )
    return [dict(shared, **per_core[c]) for c in range(NCORES)]


def kernel(
    x, W1, b1, W2, b2, Wf1, bf1, Wf2, bf2, edge_index, batch, num_graphs, _trace=False
):
    assert int(num_graphs) == NG
    meta, per_core, cnt = _preprocess(
        np.asarray(x), np.asarray(edge_index), np.asarray(batch)
    )
    nc = _get_program(meta)
    in_maps = _make_in_maps(W1, b1, W2, b2, Wf1, bf1, Wf2, bf2, per_core, cnt, meta)
    res = bass_utils.run_bass_kernel_spmd(
        nc, in_maps, core_ids=list(range(NCORES)), trace=_trace
    )
    out = np.ascontiguousarray(np.asarray(res.results[0]["out"], np.float32).T)
    if _trace:
        kernel._last_results = res
    return out
